# revision 19
# baseline (speedup 1.0000x reference)
"""2-layer GCN (DBPnet GCN head) on 8 Trainium2 NeuronCores.

Algorithm (matches the jax reference):
    x0 = relu(x)
    x1 = relu(gcn_conv(x0, W1, b1))
    x2 = gcn_conv(x1, W2, b2)
    y  = softmax(x2, axis=-1)
with gcn_conv(x) = D^-1/2 (A + I) D^-1/2 (x @ W) + b  (in-degree over dst + 1).

Sharding: nodes row-partitioned over 8 cores (6250 each); edges partitioned
by destination core so the segment-sum is core-local.  Per layer each core
computes hs = dinv * (x_shard @ W) in bf16, all-gathers hs into a full bf16
table, gathers hs[src] rows for its (dst-sorted) edges with batched indirect
DMAs, and segment-sums each 128-destination window on the tensor engine using
a one-hot selection matrix S (S[e, j] = dst_slot[e] == j) accumulated in PSUM.

Gather descriptor generation (SWDGE) runs on Q7 core pair (2q, 2q+1) for
queue q, so round-robining gather calls over 4 SWDGE queues overlaps
descriptor generation 4-way - that is the main throughput trick.

Layer 1 rows are 128 bf16 = 256B; int16 gather indices force a 2-pass
src-half split.  Layer 2 rows are 64 values; the bf16 table [N, 64] is
reinterpreted as a packed-pair table [N/2, 128] whose 256B rows hold nodes
(2k, 2k+1), indexed by src//2 (fits int16 directly, single pass); edge tiles
are parity-pure so each aggregation matmul reads the correct 64-col half.
"""

import sys

import numpy as np

sys.path.insert(0, "/opt/trn_rl_repo")

import ml_dtypes  # noqa: E402
from concourse import bass, mybir  # noqa: E402
import concourse.bacc as bacc  # noqa: E402
import concourse.tile as tile  # noqa: E402
from concourse.bass_utils import run_bass_kernel_spmd  # noqa: E402

F32 = mybir.dt.float32
BF16 = mybir.dt.bfloat16
I16 = mybir.dt.int16

C = 8            # cores
P = 128          # partitions / edge-tile size / window size
TG = 8           # edge tiles per gather DMA (dma_gather num_idxs = TG*128;
                 # >=1536 idxs per call crashes the exec unit - keep <=1024)
TB = 8           # edge tiles per S-build op
NQ = 4           # SWDGE queues (desc gen on Q7 pair (2q, 2q+1) per queue)
PAD_SLOT = 200.0  # dst_slot value for padding edges (no iota match)


def _half(N):
    """src-half split so layer-1 dma_gather int16 indices stay in range."""
    return min(25000, (N + 1) // 2)


# ---------------------------------------------------------------- host prep

def _schedule(src, dst, N):
    """Static tile schedules, shared by all cores.

    Layer 1: tile sequence [pass 0: windows 0..W-1][pass 1: ...], pass h
    covering edges with src in half h (int16-safe local indices).
    Layer 2: window-major [w0: parity-0 tiles, parity-1 tiles][w1: ...],
    indices are src//2 into the packed-pair table.
    Returns (off1 [2, W+1], off2w [W, 3], per_core) where per_core[c] =
    (si1 [P, T1*8] i16, sl1 [P, T1] bf16, si2 [P, T2*8] i16, sl2 [P, T2]).
    """
    NS = N // C
    W = (NS + P - 1) // P
    HALF = _half(N)

    order = np.argsort(dst, kind="stable")  # sorts by core then window
    s_dst = dst[order]
    s_src = src[order]

    core_bounds = np.searchsorted(s_dst, np.arange(C + 1) * NS)
    cnt1 = np.zeros((2, C, W), dtype=np.int64)
    cnt2 = np.zeros((2, C, W), dtype=np.int64)
    cores_edges = []
    for c in range(C):
        lo, hi = core_bounds[c], core_bounds[c + 1]
        d_loc = (s_dst[lo:hi] - c * NS).astype(np.int64)
        sc = s_src[lo:hi].astype(np.int64)
        h_e = (sc >= HALF).astype(np.int64)
        p_e = sc & 1
        w_e = d_loc >> 7
        for h in range(2):
            cnt1[h, c] = np.bincount(w_e[h_e == h], minlength=W)
            cnt2[h, c] = np.bincount(w_e[p_e == h], minlength=W)
        cores_edges.append((sc, d_loc, w_e, h_e, p_e))

    # L1 tiles per (pass, window), shared across cores; >=1 so psum is zeroed
    T1_hw = np.maximum(1, (cnt1.max(axis=1) + P - 1) // P)  # [2, W]
    off1 = np.zeros((2, W + 1), dtype=np.int64)
    off1[0, 1:] = np.cumsum(T1_hw[0])
    off1[1, 0] = off1[0, -1]
    off1[1, 1:] = off1[0, -1] + np.cumsum(T1_hw[1])
    T1 = int(off1[1, -1])

    # L2 tiles per (parity, window), window-major; parity-0 count >=1
    T2_pw = (cnt2.max(axis=1) + P - 1) // P  # [2, W]
    T2_pw[0] = np.maximum(T2_pw[0], 1)
    off2w = np.zeros((W, 3), dtype=np.int64)
    pos = 0
    for w in range(W):
        off2w[w, 0] = pos
        off2w[w, 1] = pos + T2_pw[0, w]
        off2w[w, 2] = pos + T2_pw[0, w] + T2_pw[1, w]
        pos = off2w[w, 2]
    T2 = int(pos)

    def _wrap_idx(si, T):
        # dma_gather index wrap: idx i -> partition i%16, col i//16,
        # replicated across the 8 groups of 16 partitions
        return np.ascontiguousarray(np.tile(si.reshape(T * 8, 16).T, (8, 1)))

    per_core = []
    for c in range(C):
        sc, d_loc, w_e, h_e, p_e = cores_edges[c]
        si1 = np.zeros(T1 * P, np.int16)
        sl1 = np.full(T1 * P, PAD_SLOT, np.float32)
        for h in range(2):
            m = h_e == h
            d_h, w_h, s_h = d_loc[m], w_e[m], sc[m]
            w_start = np.concatenate([[0], np.cumsum(cnt1[h, c])])
            rank = np.arange(len(d_h)) - w_start[w_h]
            pos = off1[h, w_h] * P + rank
            si1[pos] = (s_h - h * HALF).astype(np.int16)
            sl1[pos] = (d_h & 127).astype(np.float32)

        si2 = np.zeros(T2 * P, np.int16)
        sl2 = np.full(T2 * P, PAD_SLOT, np.float32)
        for h in range(2):
            m = p_e == h
            d_h, w_h, s_h = d_loc[m], w_e[m], sc[m]
            w_start = np.concatenate([[0], np.cumsum(cnt2[h, c])])
            rank = np.arange(len(d_h)) - w_start[w_h]
            pos = off2w[w_h, h] * P + rank
            si2[pos] = (s_h >> 1).astype(np.int16)
            sl2[pos] = (d_h & 127).astype(np.float32)

        per_core.append((
            _wrap_idx(si1, T1),
            np.ascontiguousarray(sl1.reshape(T1, P).T.astype(ml_dtypes.bfloat16)),
            _wrap_idx(si2, T2),
            np.ascontiguousarray(sl2.reshape(T2, P).T.astype(ml_dtypes.bfloat16)),
        ))
    return off1, off2w, per_core


# ------------------------------------------------------------- device build

def build_program(nc, N, H, F1, F2, off1, off2w, cc=True):
    """Emit the SPMD program. All cores run identical code; per-core data
    comes in through the input tensors."""
    NS = N // C
    W = (NS + P - 1) // P
    NSP = W * P
    HALF = _half(N)
    T1 = int(off1[1, -1])
    T2 = int(off2w[-1, 2])

    # ---- I/O -------------------------------------------------------------
    d_xT = nc.dram_tensor("xT", [H, NSP], F32, kind="ExternalInput")
    d_W1 = nc.dram_tensor("W1", [H, F1], BF16, kind="ExternalInput")
    d_W2 = nc.dram_tensor("W2", [F1, F2], BF16, kind="ExternalInput")
    d_b1 = nc.dram_tensor("b1r", [P, F1], F32, kind="ExternalInput")
    d_b2 = nc.dram_tensor("b2r", [P, F2], F32, kind="ExternalInput")
    d_dinv = nc.dram_tensor("dinv", [P, W], F32, kind="ExternalInput")
    d_iota = nc.dram_tensor("iota", [P, P], BF16, kind="ExternalInput")
    d_ident = nc.dram_tensor("ident", [P, P], BF16, kind="ExternalInput")
    d_si1 = nc.dram_tensor("srcidx1", [P, T1 * 8], I16, kind="ExternalInput")
    d_sl1 = nc.dram_tensor("dstslot1", [P, T1], BF16, kind="ExternalInput")
    d_si2 = nc.dram_tensor("srcidx2", [P, T2 * 8], I16, kind="ExternalInput")
    d_sl2 = nc.dram_tensor("dstslot2", [P, T2], BF16, kind="ExternalInput")
    d_y = nc.dram_tensor("y", [NS, F2], F32, kind="ExternalOutput")

    with tile.TileContext(nc) as tc:
        with (
            tc.tile_pool(name="const", bufs=1) as const_pool,
            tc.tile_pool(name="persist", bufs=1) as persist,
            tc.tile_pool(name="gath", bufs=8) as gath_pool,
            tc.tile_pool(name="sbuild", bufs=12) as s_pool,
            tc.tile_pool(name="winbuf", bufs=3) as win_pool,
            tc.tile_pool(name="small", bufs=6) as small_pool,
            tc.tile_pool(name="agg", bufs=2, space="PSUM") as psum_agg,
            tc.tile_pool(name="dense", bufs=2, space="PSUM") as psum_dense,
            tc.tile_pool(name="tpose", bufs=2, space="PSUM") as psum_t,
            tc.tile_pool(name="dram", bufs=1, space="DRAM") as dram,
        ):
            # ---- constants / persistent state -----------------------------
            sb_W1 = const_pool.tile([H, F1], BF16, tag="w1")
            nc.sync.dma_start(out=sb_W1[:], in_=d_W1[:])
            sb_W2 = const_pool.tile([F1, F2], BF16, tag="w2")
            nc.sync.dma_start(out=sb_W2[:], in_=d_W2[:])
            sb_b1 = const_pool.tile([P, F1], F32, tag="b1")
            nc.sync.dma_start(out=sb_b1[:], in_=d_b1[:])
            sb_b2 = const_pool.tile([P, F2], F32, tag="b2")
            nc.sync.dma_start(out=sb_b2[:], in_=d_b2[:])
            sb_dinv = const_pool.tile([P, W], F32, tag="dinv")
            nc.sync.dma_start(out=sb_dinv[:], in_=d_dinv[:])
            sb_iota = const_pool.tile([P, P], BF16, tag="iota")
            nc.sync.dma_start(out=sb_iota[:], in_=d_iota[:])
            sb_ident = const_pool.tile([P, P], BF16, tag="ident")
            nc.sync.dma_start(out=sb_ident[:], in_=d_ident[:])
            sb_si1 = const_pool.tile([P, T1 * 8], I16, tag="srcidx1")
            nc.sync.dma_start(out=sb_si1[:], in_=d_si1[:])
            sb_sl1 = const_pool.tile([P, T1], BF16, tag="dstslot1")
            nc.sync.dma_start(out=sb_sl1[:], in_=d_sl1[:])
            sb_si2 = const_pool.tile([P, T2 * 8], I16, tag="srcidx2")
            nc.sync.dma_start(out=sb_si2[:], in_=d_si2[:])
            sb_sl2 = const_pool.tile([P, T2], BF16, tag="dstslot2")
            nc.sync.dma_start(out=sb_sl2[:], in_=d_sl2[:])

            sb_xT = persist.tile([H, NSP], F32, tag="xT")
            nc.sync.dma_start(out=sb_xT[:], in_=d_xT[:])
            sb_x0b = persist.tile([H, NSP], BF16, tag="x0b")
            sb_hs1 = persist.tile([P, W, F1], BF16, tag="hs1")
            sb_hs2 = persist.tile([P, W, F2], BF16, tag="hs2")
            # pass-0 partial aggregates; acc1 reuses the xT slot (dead then)
            sb_acc1 = persist.tile([P, W, F1], F32, tag="xT")

            # DRAM bounce + gather tables (hs2 packed-pair view: [N/2, 2*F2])
            hs1_loc = dram.tile([NS, F1], BF16, tag="hs1_loc")
            hs1_full = dram.tile([N, F1], BF16, tag="hs1_full",
                                 addr_space="Shared")
            hs2_loc = dram.tile([NS // 2, 2 * F2], BF16, tag="hs2_loc")
            hs2_full = dram.tile([N // 2, 2 * F2], BF16, tag="hs2_full",
                                 addr_space="Shared")

            gcall = [0]  # gather-call counter for SWDGE queue round-robin

            # ---- phase 1: x0 = relu(x) bf16; hs1 = dinv * (x0 @ W1) -------
            nc.scalar.activation(sb_x0b[:], sb_xT[:],
                                 mybir.ActivationFunctionType.Relu)
            for w in range(W):
                rows = min(P, NS - w * P)
                ph = psum_dense.tile([P, F1], F32, tag="dense")
                nc.tensor.matmul(ph[:], lhsT=sb_x0b[:, w * P:(w + 1) * P],
                                 rhs=sb_W1[:], start=True, stop=True)
                nc.scalar.activation(sb_hs1[:, w, :], ph[:],
                                     mybir.ActivationFunctionType.Copy,
                                     scale=sb_dinv[:, w:w + 1])
                nc.sync.dma_start(out=hs1_loc[w * P:w * P + rows, :],
                                  in_=sb_hs1[:rows, w, :])

            # ---- phase 2: all-gather layer-1 table ------------------------
            if cc:
                nc.gpsimd.collective_compute(
                    "AllGather", mybir.AluOpType.bypass,
                    replica_groups=[list(range(C))],
                    ins=[hs1_loc[:].opt()], outs=[hs1_full[:].opt()])
            else:  # timeline-sim stand-in with the same dependency shape
                nc.sync.dma_start(out=hs1_full[:NS, :], in_=hs1_loc[:])

            # ---- S-matrix builder (shared) --------------------------------
            def build_s(sb_sl, T):
                sts = {}
                for t0 in range(0, T, TB):
                    n = min(TB, T - t0)
                    s = s_pool.tile([P, TB, P], BF16, tag="sbuild")
                    nc.vector.tensor_tensor(
                        out=s[:, :n, :],
                        in0=sb_sl[:, t0:t0 + n].to_broadcast([P, n, P]),
                        in1=sb_iota[:].rearrange(
                            "p (o n) -> p o n", o=1).to_broadcast([P, n, P]),
                        op=mybir.AluOpType.is_equal)
                    sts[t0 // TB] = s
                return sts

            # ---- layer-1 edge aggregation (2 passes over src halves) ------
            def edge_layer1(out_cb):
                sts = build_s(sb_sl1, T1)
                gts = {}
                for h in range(2):
                    p_lo, p_hi = int(off1[h, 0]), int(off1[h, -1])
                    tab = hs1_full[h * HALF:(h + 1) * HALF, :]
                    for w in range(W):
                        t0w, t1w = int(off1[h, w]), int(off1[h, w + 1])
                        pa = psum_agg.tile([P, F1], F32, tag="agg")
                        for t in range(t0w, t1w):
                            if (t - p_lo) % TG == 0:
                                g = gath_pool.tile([P, TG, F1], BF16,
                                                   tag="gath")
                                n = min(TG, p_hi - t)
                                nc.gpsimd.dma_gather(
                                    g[:, :n, :], tab,
                                    sb_si1[:, t * 8:(t + n) * 8],
                                    n * P, n * P, F1,
                                    queue_num=gcall[0] % NQ)
                                gcall[0] += 1
                                gts[(t - p_lo) // TG + 1000 * h] = g
                            nc.tensor.matmul(
                                pa[:],
                                lhsT=sts[t // TB][:, t % TB, :],
                                rhs=gts[(t - p_lo) // TG + 1000 * h][
                                    :, (t - p_lo) % TG, :],
                                start=(t == t0w), stop=(t == t1w - 1))
                        rows = min(P, NS - w * P)
                        if h == 0:
                            nc.scalar.copy(sb_acc1[:, w, :], pa[:])
                        else:
                            out_cb(w, rows, pa[:])

            # ---- layer-1 epilogue: relu, transpose, dense L2 --------------
            def l1_out(w, rows, pa):
                tmp = win_pool.tile([P, F1], F32, tag="tmp")
                nc.vector.tensor_tensor(out=tmp[:], in0=pa,
                                        in1=sb_acc1[:, w, :],
                                        op=mybir.AluOpType.add)
                nc.vector.tensor_tensor(out=tmp[:], in0=tmp[:],
                                        in1=sb_hs1[:, w, :],
                                        op=mybir.AluOpType.add)
                xs = win_pool.tile([P, F1], F32, tag="xs")
                nc.scalar.activation(xs[:], tmp[:],
                                     mybir.ActivationFunctionType.Copy,
                                     scale=sb_dinv[:, w:w + 1])
                nc.vector.tensor_tensor(out=xs[:], in0=xs[:], in1=sb_b1[:],
                                        op=mybir.AluOpType.add)
                x1 = win_pool.tile([P, F1], BF16, tag="x1")
                nc.scalar.activation(x1[:], xs[:],
                                     mybir.ActivationFunctionType.Relu)
                # transpose x1 -> lhsT for the layer-2 dense matmul
                pt = psum_t.tile([P, P], BF16, tag="tpose")
                nc.tensor.transpose(pt[:], x1[:], sb_ident[:])
                x1T = win_pool.tile([P, P], BF16, tag="x1T")
                nc.vector.tensor_copy(x1T[:], pt[:])
                ph2 = psum_dense.tile([P, F1], F32, tag="dense")
                nc.tensor.matmul(ph2[:, :F2], lhsT=x1T[:], rhs=sb_W2[:],
                                 start=True, stop=True)
                nc.scalar.activation(sb_hs2[:, w, :], ph2[:, :F2],
                                     mybir.ActivationFunctionType.Copy,
                                     scale=sb_dinv[:, w:w + 1])
                # packed-pair DRAM write: [128 part, F2] -> [rows//2, 2*F2]
                nc.sync.dma_start(
                    out=hs2_loc[w * 64:w * 64 + rows // 2, :],
                    in_=sb_hs2[:rows, w, :])

            edge_layer1(l1_out)

            # ---- phase 5: all-gather layer-2 packed table -----------------
            if cc:
                nc.gpsimd.collective_compute(
                    "AllGather", mybir.AluOpType.bypass,
                    replica_groups=[list(range(C))],
                    ins=[hs2_loc[:].opt()], outs=[hs2_full[:].opt()])
            else:
                nc.sync.dma_start(out=hs2_full[:NS // 2, :], in_=hs2_loc[:])

            # ---- phase 6: layer-2 edges + softmax (single pass) -----------
            def l2_out(w, rows, pa):
                t0 = win_pool.tile([P, F2], F32, tag="tmp2a")
                nc.vector.tensor_tensor(out=t0[:], in0=pa,
                                        in1=sb_hs2[:, w, :],
                                        op=mybir.AluOpType.add)
                tmp = win_pool.tile([P, F2], F32, tag="tmp2")
                nc.scalar.activation(tmp[:], t0[:],
                                     mybir.ActivationFunctionType.Copy,
                                     scale=sb_dinv[:, w:w + 1])
                nc.vector.tensor_tensor(out=tmp[:], in0=tmp[:], in1=sb_b2[:],
                                        op=mybir.AluOpType.add)
                nmax = small_pool.tile([P, 1], F32, tag="nmax")
                nc.vector.tensor_reduce(nmax[:], tmp[:],
                                        axis=mybir.AxisListType.X,
                                        op=mybir.AluOpType.max, negate=True)
                ex = win_pool.tile([P, F2], F32, tag="ex")
                ssum = small_pool.tile([P, 1], F32, tag="ssum")
                nc.scalar.activation(ex[:], tmp[:],
                                     mybir.ActivationFunctionType.Exp,
                                     bias=nmax[:], accum_out=ssum[:])
                rsum = small_pool.tile([P, 1], F32, tag="rsum")
                nc.vector.reciprocal(rsum[:], ssum[:])
                yw = win_pool.tile([P, F2], F32, tag="yw")
                nc.scalar.activation(yw[:], ex[:],
                                     mybir.ActivationFunctionType.Copy,
                                     scale=rsum[:])
                nc.sync.dma_start(out=d_y[w * P:w * P + rows, :],
                                  in_=yw[:rows, :])

            sts2 = build_s(sb_sl2, T2)
            gts2 = {}
            for w in range(W):
                t0w, t2w = int(off2w[w, 0]), int(off2w[w, 2])
                t1w = int(off2w[w, 1])  # parity boundary
                pa = psum_agg.tile([P, F2], F32, tag="agg2")
                for t in range(t0w, t2w):
                    if t % TG == 0:
                        g = gath_pool.tile([P, TG, 2 * F2], BF16, tag="gath")
                        n = min(TG, T2 - t)
                        nc.gpsimd.dma_gather(
                            g[:, :n, :], hs2_full[:],
                            sb_si2[:, t * 8:(t + n) * 8],
                            n * P, n * P, 2 * F2,
                            queue_num=gcall[0] % NQ)
                        gcall[0] += 1
                        gts2[t // TG] = g
                    par = 0 if t < t1w else 1
                    nc.tensor.matmul(
                        pa[:],
                        lhsT=sts2[t // TB][:, t % TB, :],
                        rhs=gts2[t // TG][:, t % TG,
                                          par * F2:(par + 1) * F2],
                        start=(t == t0w), stop=(t == t2w - 1))
                rows = min(P, NS - w * P)
                l2_out(w, rows, pa[:])

    return {
        "in_names": ["xT", "W1", "W2", "b1r", "b2r", "dinv", "iota", "ident",
                     "srcidx1", "dstslot1", "srcidx2", "dstslot2"],
        "out_name": "y",
    }


# ---------------------------------------------------------------- frontend

_CACHE = {}


def _build_and_compile(N, H, F1, F2, off1, off2w):
    nc = bacc.Bacc("TRN2", target_bir_lowering=False, debug=False,
                   enable_asserts=False, num_devices=C,
                   num_swdge_queues=NQ, dynamic_dma_scratch_size=32768)
    meta = build_program(nc, N, H, F1, F2, off1, off2w)
    nc.compile()
    return nc, meta


def prepare_inputs(x, edge_index, W1, b1, W2, b2):
    N, H = x.shape
    F1 = W1.shape[1]
    F2 = W2.shape[1]
    NS = N // C
    W = (NS + P - 1) // P
    NSP = W * P

    src = np.asarray(edge_index[0], dtype=np.int64)
    dst = np.asarray(edge_index[1], dtype=np.int64)
    deg = np.bincount(dst, minlength=N).astype(np.float32) + 1.0
    dinv = (1.0 / np.sqrt(deg)).astype(np.float32)

    off1, off2w, per_core = _schedule(src, dst, N)

    iota = np.ascontiguousarray(
        np.tile(np.arange(P, dtype=np.float32), (P, 1)).astype(
            ml_dtypes.bfloat16))
    ident = np.eye(P, dtype=np.float32).astype(ml_dtypes.bfloat16)
    b1r = np.ascontiguousarray(np.tile(np.asarray(b1, np.float32), (P, 1)))
    b2r = np.ascontiguousarray(np.tile(np.asarray(b2, np.float32), (P, 1)))
    W1b = np.ascontiguousarray(np.asarray(W1, np.float32).astype(
        ml_dtypes.bfloat16))
    W2b = np.ascontiguousarray(np.asarray(W2, np.float32).astype(
        ml_dtypes.bfloat16))

    in_maps = []
    for c in range(C):
        xs = np.zeros((NSP, H), np.float32)
        xs[:NS] = np.asarray(x[c * NS:(c + 1) * NS], np.float32)
        xT = np.ascontiguousarray(xs.T)
        dv = np.ones(NSP, np.float32)
        dv[:NS] = dinv[c * NS:(c + 1) * NS]
        dv = np.ascontiguousarray(dv.reshape(W, P).T)
        si1, sl1, si2, sl2 = per_core[c]
        in_maps.append({
            "xT": xT, "W1": W1b, "W2": W2b, "b1r": b1r, "b2r": b2r,
            "dinv": dv, "iota": iota, "ident": ident,
            "srcidx1": si1, "dstslot1": sl1,
            "srcidx2": si2, "dstslot2": sl2,
        })
    return in_maps, (N, H, F1, F2, off1, off2w)


def kernel(x, edge_index, W1, b1, W2, b2, trace=False):
    x = np.asarray(x)
    in_maps, key = prepare_inputs(x, edge_index, W1, b1, W2, b2)
    N, H, F1, F2, off1, off2w = key
    ck = (N, H, F1, F2, off1.tobytes(), off2w.tobytes())
    if ck not in _CACHE:
        _CACHE.clear()
        _CACHE[ck] = _build_and_compile(N, H, F1, F2, off1, off2w)
    nc, meta = _CACHE[ck]
    res = run_bass_kernel_spmd(nc, in_maps, core_ids=list(range(C)),
                               trace=trace)
    y = np.concatenate([res.results[c]["y"] for c in range(C)], axis=0)
    if trace:
        kernel.last_exec_time_ns = res.exec_time_ns
    return y.astype(np.float32)


kernel.last_exec_time_ns = None
